# revision 19
# baseline (speedup 1.0000x reference)
"""GRU cell kernel for Trainium2, data-parallel across 8 NeuronCores.

Reference computation (per batch row):
    concat = [h_prev, x]                       # [B, 2048]
    z = sigmoid(concat @ W_z.T + b_z)          # [B, 1024]
    r = sigmoid(concat @ W_r.T + b_r)
    h_tilde = tanh([r*h_prev, x] @ W_h.T + b_h)
    h = (1-z)*h_prev + z*h_tilde

Sharding: batch dim (8192) split 1024/core; weights replicated.
Layout on device is feature-major; batch is the matmul moving dimension,
hidden units the PSUM partition dim. Host transposes in/out.

Matmuls run in fp8-e4m3 with perf_mode=DoubleRow (2 contraction rows per
PE cell). The PE moving port feeds 2 bytes/partition/cycle, so a DR
matmul streams a [256 x 512-batch] contraction chunk in ~512 cycles --
2x the flops of bf16 per cycle; measured ~216ns/MM = ~155 TF/s, the fp8
roofline. Weights are host-scaled by 512 so |w|<=11.3 sits in e4m3's
normal range (raw |w|<=0.022 is subnormal); the activation instruction's
scale operand undoes it for free.

Activations are host-swizzled to [partition, batch-half, feature-chunk,
512] so every DMA moves 4KB-contiguous runs per partition (128
descriptors/transfer instead of 1024 512B ones).

mode:
  fp8h  - all three gates fp8-DR.           (HW rel_fro ~1.76e-2)
  split - r/z fp8-DR; h-gate h-part fp8-DR over r*h_prev, x-part bf16.
                                            (sim rel_fro ~1.25e-2)
"""

import numpy as np

import concourse.bacc as bacc
import concourse.bass as bass
import concourse.mybir as mybir
import concourse.tile as tile
from concourse import bass_utils

P = 128
B = 8192
I = 1024
H = 1024
K = I + H            # 2048 contraction
NCORES = 8
BS = B // NCORES     # 1024 batch rows per core
MT = H // P          # 8 m-tiles (hidden units)
KT = K // P          # 16 k-chunks of 128
KK = K // (2 * P)    # 8 double-chunks of 256 (DoubleRow)
NFREE = 512          # moving free dim (one PSUM bank of fp32)
NT = BS // NFREE     # 2 n-tiles per core
KO = 8               # feature chunks per 1024-feature tensor
WS = 512.0           # host-side weight scale for fp8 range

F32 = mybir.dt.float32
BF16 = mybir.dt.bfloat16
F8 = mybir.dt.float8e4

AF = mybir.ActivationFunctionType
DR = mybir.MatmulPerfMode.DoubleRow


def build_kernel(mode: str = "fp8h"):
    """Build the per-core Bass kernel. Returns compiled nc."""
    assert mode in ("fp8h", "split")
    split = mode == "split"
    nc = bacc.Bacc("TRN2", target_bir_lowering=False, debug=False)

    # DRAM I/O (per-core shapes). Activations are pre-swizzled on the host
    # to [P, NT*KO*NFREE] so each partition's bytes are contiguous.
    AW = NT * KO * NFREE
    x8 = nc.dram_tensor("x8", [P, AW], F8, kind="ExternalInput").ap()
    h8 = nc.dram_tensor("h8", [P, AW], F8, kind="ExternalInput").ap()
    hb = nc.dram_tensor("hb", [P, AW], BF16, kind="ExternalInput").ap()
    Wr = nc.dram_tensor("Wr", [MT, P, K], F8, kind="ExternalInput").ap()
    Wz = nc.dram_tensor("Wz", [MT, P, K], F8, kind="ExternalInput").ap()
    if split:
        xb = nc.dram_tensor("xb", [P, AW], BF16, kind="ExternalInput").ap()
        Whh = nc.dram_tensor("Whh", [MT, P, H], F8, kind="ExternalInput").ap()
        Whx = nc.dram_tensor("Whx", [MT, P, I], BF16,
                             kind="ExternalInput").ap()
    else:
        Wh = nc.dram_tensor("Wh", [MT, P, K], F8, kind="ExternalInput").ap()
    bz = nc.dram_tensor("bz", [P, MT], F32, kind="ExternalInput").ap()
    br = nc.dram_tensor("br", [P, MT], F32, kind="ExternalInput").ap()
    bh = nc.dram_tensor("bh", [P, MT], F32, kind="ExternalInput").ap()
    out = nc.dram_tensor("out", [H, BS], BF16, kind="ExternalOutput").ap()

    with tile.TileContext(nc) as tc:
        with (
            tc.tile_pool(name="acts", bufs=1) as acts,
            tc.tile_pool(name="gates", bufs=1) as gates,
            tc.tile_pool(name="wpool", bufs=1) as wpool,
            tc.tile_pool(name="opool", bufs=10) as opool,
            tc.tile_pool(name="ppool", bufs=8, space="PSUM") as ppool,
        ):
            bz_sb = acts.tile([P, MT], F32)
            br_sb = acts.tile([P, MT], F32)
            bh_sb = acts.tile([P, MT], F32)

            # Weight tiles, [P, KT, P]: [:, 2k:2k+2, :] is a DoubleRow
            # stationary operand [128, 2, 128].
            wr_sb = [wpool.tile([P, KT, P], F8, name=f"wr{m}")
                     for m in range(MT)]
            wz_sb = [wpool.tile([P, KT, P], F8, name=f"wz{m}")
                     for m in range(MT)]
            if split:
                whh_sb = [wpool.tile([P, KT // 2, P], F8, name=f"whh{m}")
                          for m in range(MT)]
                whx_sb = [wpool.tile([P, I], BF16, name=f"whx{m}")
                          for m in range(MT)]
            else:
                wh_sb = [wpool.tile([P, KT, P], F8, name=f"wh{m}")
                         for m in range(MT)]

            # Pre-warm the ACT sigmoid table during the DMA fill.
            warm = acts.tile([P, 1], F32)
            nc.vector.memset(warm[:], 0.0)
            nc.scalar.activation(warm[:], warm[:], AF.Sigmoid)

            # Persistent activations: [p, n-half, ko, bw]
            x8_sb = acts.tile([P, NT, KO, NFREE], F8)
            h8_sb = acts.tile([P, NT, KO, NFREE], F8)
            hb_sb = acts.tile([P, NT, KO, NFREE], BF16)
            xb_sb = (acts.tile([P, NT, KO, NFREE], BF16, name="xb_sb")
                     if split else None)

            def half(dram, n):
                return dram[:, n * KO * NFREE:(n + 1) * KO * NFREE]

            # Head DMA schedule. The h-half of the acts rides the sync ring
            # and the x-half the scalar ring, so the mt0 chains' kk0-3
            # (h_prev) and kk4-7 (x) payloads stream in parallel instead of
            # serializing on one HWDGE ring (the early window is chip-HBM
            # contended -- all 8 cores load at once). The gpsimd SWDGE queue
            # (~1us extra latency, otherwise idle) takes everything needed
            # later than ~15us: remaining weights, then hb, then wz/wh.
            q0 = 2 * NFREE  # first two feature-chunks of a half
            nc.sync.dma_start(h8_sb[:, 0, 0:2, :], h8[:, 0:q0])
            nc.sync.dma_start(h8_sb[:, 0, 2:, :], h8[:, q0:KO * NFREE])
            nc.sync.dma_start(h8_sb[:, 1], half(h8, 1))
            nc.scalar.dma_start(br_sb[:], br)
            nc.scalar.dma_start(x8_sb[:, 0], half(x8, 0))
            nc.scalar.dma_start(x8_sb[:, 1], half(x8, 1))
            nc.scalar.dma_start(bz_sb[:], bz)
            nc.scalar.dma_start(bh_sb[:], bh)
            # wr0/wr1 are needed first/soon; everything after them is bulk.
            # SDMA engines round-robin between queues at packet granularity,
            # so un-gated bulk on the gpsimd ring would steal ~1/3 of the
            # chip-contended head bandwidth from the critical act loads
            # above. The dummy copy below reads from the x8 n1 half, so the
            # tile framework holds the bulk descriptors back until the last
            # critical act DMA has landed.
            nc.gpsimd.dma_start(wr_sb[0][:], Wr[0])
            nc.gpsimd.dma_start(wr_sb[1][:], Wr[1])
            dma_gate = opool.tile([P, 8], F8, name="dma_gate")
            nc.gpsimd.tensor_copy(dma_gate[:], x8_sb[:, 1, 0, 0:8])
            for m in range(2, MT):
                nc.gpsimd.dma_start(wr_sb[m][:], Wr[m])
            for n in range(NT):
                nc.gpsimd.dma_start(hb_sb[:, n], half(hb, n))
            for m in range(MT):
                nc.gpsimd.dma_start(wz_sb[m][:], Wz[m])
            if split:
                for n in range(NT):
                    nc.scalar.dma_start(xb_sb[:, n], half(xb, n))
                for m in range(MT):
                    nc.gpsimd.dma_start(whh_sb[m][:], Whh[m])
                for m in range(MT):
                    nc.gpsimd.dma_start(whx_sb[m][:], Whx[m])
            else:
                for m in range(MT):
                    nc.gpsimd.dma_start(wh_sb[m][:], Wh[m])

            # Gate results, same swizzled layout
            z_sb = gates.tile([P, NT, KO, NFREE], BF16)
            rh_sb = gates.tile([P, NT, KO, NFREE], F8)

            # Warm the PE while the first acts stream in: ~3.4us of dummy
            # matmuls on a zeroed tile un-throttle the HAM clock gate
            # (1.2 -> 2.4 GHz needs a busy activity window), so the first
            # real chains run at full rate instead of paying the cold ramp.
            zt = acts.tile([P, 2, NFREE // 2], F8)
            nc.vector.memset(zt[:], 0.0)
            ps_wf = ppool.tile([P, NFREE], F32, tag="ps", name="ps_warm")
            for i in range(16):
                nc.tensor.matmul(ps_wf[:, 0:NFREE // 2], zt[:, :, 0:P], zt[:],
                                 start=(i == 0), stop=(i == 15), perf_mode=DR)

            def rz_rhs(kk, n):
                """fp8 moving operand [128,2,512] for concat chunk kk."""
                if kk < KK // 2:
                    return h8_sb[:, n, 2 * kk:2 * kk + 2, :]
                c = kk - KK // 2
                return x8_sb[:, n, 2 * c:2 * c + 2, :]

            def h_rhs(kk, n):
                """fp8 moving operand for the h-gate ([r*h_prev, x])."""
                if kk < KK // 2:
                    return rh_sb[:, n, 2 * kk:2 * kk + 2, :]
                c = kk - KK // 2
                return x8_sb[:, n, 2 * c:2 * c + 2, :]

            def finish(stage, mt, n, ps, width=NFREE, sub=0):
                """PSUM -> activation -> elementwise -> (store)."""
                lo, hi = sub * width, (sub + 1) * width
                if stage == "r":
                    r_tmp = opool.tile([P, width], BF16, tag="rt")
                    nc.scalar.activation(r_tmp, ps, AF.Sigmoid,
                                         bias=br_sb[:, mt:mt + 1],
                                         scale=1.0 / WS)
                    nc.vector.tensor_mul(
                        rh_sb[:, n, mt, lo:hi], r_tmp, hb_sb[:, n, mt, lo:hi])
                elif stage == "z":
                    nc.scalar.activation(z_sb[:, n, mt, lo:hi], ps,
                                         AF.Sigmoid,
                                         bias=bz_sb[:, mt:mt + 1],
                                         scale=1.0 / WS)
                else:  # h = h_prev + z*(h_tilde - h_prev)
                    hpv = hb_sb[:, n, mt, lo:hi]
                    ht = opool.tile([P, width], BF16, tag="ht")
                    nc.scalar.activation(ht, ps, AF.Tanh,
                                         bias=bh_sb[:, mt:mt + 1],
                                         scale=1.0 / WS)
                    nc.vector.tensor_sub(ht, ht, hpv)
                    nc.vector.tensor_mul(ht, ht, z_sb[:, n, mt, lo:hi])
                    nc.vector.tensor_add(ht, ht, hpv)
                    ns = slice(n * NFREE + lo, n * NFREE + hi)
                    nc.sync.dma_start(out[mt * P:(mt + 1) * P, ns], ht)

            def chain(stage, w_sb, rhs, mt, n, nsub=1, nchain=1):
                """One (mt, n) PSUM accumulation chain + its epilogue.

                LDWEIGHTS is emitted 1:1 per matmul by the compiler and at
                ~135ns hides under the ~216ns moving-port-bound DR matmul
                stream, so plain k-sequential chains already run at the
                roofline; chain order only needs to match DMA arrival order.
                nchain>1 splits the matmuls into narrower column chains so
                the epilogue of chain c pipelines under chain c+1's matmuls
                (used for the very last group to shrink the kernel tail).
                """
                wc = NFREE // nchain
                for c in range(nchain):
                    psf = ppool.tile([P, NFREE], F32, tag="ps",
                                     name=f"ps_{stage}{mt}_{n}_{c}")
                    ps = psf[:, 0:wc]
                    for kk in range(KK):
                        nc.tensor.matmul(
                            ps, w_sb[mt][:, 2 * kk:2 * kk + 2, :],
                            rhs(kk, n)[:, :, c * wc:(c + 1) * wc],
                            start=(kk == 0), stop=(kk == KK - 1),
                            perf_mode=DR)
                    w2 = wc // nsub
                    for s in range(nsub):
                        finish(stage, mt, n, ps[:, s * w2:(s + 1) * w2],
                               width=w2, sub=c * nsub + s)

            for mt in range(MT):
                for n in range(NT):
                    chain("r", wr_sb, rz_rhs, mt, n)
            for mt in range(MT):
                for n in range(NT):
                    chain("z", wz_sb, rz_rhs, mt, n)

            if not split:
                for mt in range(MT):
                    for n in range(NT):
                        last = mt == MT - 1 and n == NT - 1
                        chain("h", wh_sb, h_rhs, mt, n,
                              nsub=1 if last else (2 if mt == MT - 1 else 1),
                              nchain=2 if last else 1)
            else:
                for mt in range(MT):
                    for n in range(NT):
                        ps = ppool.tile([P, NFREE], F32, tag="ps",
                                        name=f"ps_h{mt}_{n}")
                        for kc in range(KT // 2):
                            nc.tensor.matmul(
                                ps, whx_sb[mt][:, kc * P:(kc + 1) * P],
                                xb_sb[:, n, kc, :],
                                start=(kc == 0), stop=False)
                        for kk in range(KK // 2):
                            nc.tensor.matmul(
                                ps, whh_sb[mt][:, 2 * kk:2 * kk + 2, :],
                                rh_sb[:, n, 2 * kk:2 * kk + 2, :],
                                start=False, stop=(kk == KK // 2 - 1),
                                perf_mode=DR)
                        last = mt == MT - 1
                        nsub = 4 if (last and n == NT - 1) else (
                            2 if last else 1)
                        w2 = NFREE // nsub
                        for s in range(nsub):
                            finish("h", mt, n, ps[:, s * w2:(s + 1) * w2],
                                   width=w2, sub=s)

    nc.compile()
    return nc


def _prep_inputs(x, h_prev, W_z, b_z, W_r, b_r, W_h, b_h, mode="fp8h"):
    """Host-side relayout: swizzled feature-major acts, m-tiled weights."""
    import ml_dtypes
    F8NP = ml_dtypes.float8_e4m3fn
    BFNP = ml_dtypes.bfloat16
    split = mode == "split"

    def prep_w(W, dt):
        # w[mt, p, ko*128+m] = W[mt*128+m, ko*128+p], scaled for fp8 range
        MTl, Kl = W.shape[0] // P, W.shape[1]
        W4 = (W * WS).reshape(MTl, P, Kl // P, P)      # [mt, m, ko, p]
        return np.ascontiguousarray(
            W4.transpose(0, 3, 2, 1)).reshape(MTl, P, Kl).astype(dt)

    def prep_act(aT, dt):
        # [F, bs] -> [p, n, ko, bw] -> flat [P, AW]
        a4 = aT.reshape(KO, P, NT, NFREE).transpose(1, 2, 0, 3)
        return np.ascontiguousarray(a4).reshape(P, NT * KO * NFREE).astype(dt)

    def prep_b(b):
        return np.ascontiguousarray(b.reshape(MT, P).T)

    xT = np.ascontiguousarray(x.T)                         # [I, B] f32
    hT = np.ascontiguousarray(h_prev.T)                    # [H, B] f32
    shared = {
        "Wr": prep_w(W_r, F8NP), "Wz": prep_w(W_z, F8NP),
        "bz": prep_b(b_z), "br": prep_b(b_r), "bh": prep_b(b_h),
    }
    if split:
        shared["Whh"] = prep_w(W_h[:, :H], F8NP)
        shared["Whx"] = prep_w(W_h[:, H:], BFNP)
    else:
        shared["Wh"] = prep_w(W_h, F8NP)
    in_maps = []
    for c in range(NCORES):
        bs = slice(c * BS, (c + 1) * BS)
        m = dict(shared)
        m["x8"] = prep_act(xT[:, bs], F8NP)
        m["h8"] = prep_act(hT[:, bs], F8NP)
        m["hb"] = prep_act(hT[:, bs], BFNP)
        if split:
            m["xb"] = prep_act(xT[:, bs], BFNP)
        in_maps.append(m)
    return in_maps


def run(inputs, mode="fp8h", trace=False, **run_kwargs):
    """Compile + run on 8 cores. Returns (output [B,H] f32, results)."""
    run_kwargs.pop("mm_dtype", None)
    nc = build_kernel(mode)
    in_maps = _prep_inputs(**inputs, mode=mode)
    res = bass_utils.run_bass_kernel_spmd(
        nc, in_maps, core_ids=list(range(NCORES)), trace=trace, **run_kwargs)
    outT = np.concatenate(
        [res.results[c]["out"] for c in range(NCORES)], axis=1)  # [H, B] bf16
    return np.ascontiguousarray(outT.T).astype(np.float32), res


def kernel(**inputs) -> np.ndarray:
    import time as _time
    try:
        out, _ = run(inputs)
    except Exception:
        # The axon-tunneled device occasionally reports a transient
        # "unrecoverable" state right after a crashed session; a fresh
        # attempt after a short pause recovers.
        _time.sleep(15)
        out, _ = run(inputs)
    return out


# revision 21
# speedup vs baseline: 1.0233x; 1.0233x over previous
"""GRU cell kernel for Trainium2, data-parallel across 8 NeuronCores.

Reference computation (per batch row):
    concat = [h_prev, x]                       # [B, 2048]
    z = sigmoid(concat @ W_z.T + b_z)          # [B, 1024]
    r = sigmoid(concat @ W_r.T + b_r)
    h_tilde = tanh([r*h_prev, x] @ W_h.T + b_h)
    h = (1-z)*h_prev + z*h_tilde

Sharding: batch dim (8192) split 1024/core; weights replicated.
Layout on device is feature-major; batch is the matmul moving dimension,
hidden units the PSUM partition dim. Host transposes in/out.

Matmuls run in fp8-e4m3 with perf_mode=DoubleRow (2 contraction rows per
PE cell). The PE moving port feeds 2 bytes/partition/cycle, so a DR
matmul streams a [256 x 512-batch] contraction chunk in ~512 cycles --
2x the flops of bf16 per cycle; measured ~216ns/MM = ~155 TF/s, the fp8
roofline. Weights are host-scaled by 512 so |w|<=11.3 sits in e4m3's
normal range (raw |w|<=0.022 is subnormal); the activation instruction's
scale operand undoes it for free.

Activations are host-swizzled to [partition, batch-half, feature-chunk,
512] so every DMA moves 4KB-contiguous runs per partition (128
descriptors/transfer instead of 1024 512B ones).

mode:
  fp8h  - all three gates fp8-DR.           (HW rel_fro ~1.76e-2)
  split - r/z fp8-DR; h-gate h-part fp8-DR over r*h_prev, x-part bf16.
                                            (sim rel_fro ~1.25e-2)
"""

import numpy as np

import concourse.bacc as bacc
import concourse.bass as bass
import concourse.mybir as mybir
import concourse.tile as tile
from concourse import bass_utils

P = 128
B = 8192
I = 1024
H = 1024
K = I + H            # 2048 contraction
NCORES = 8
BS = B // NCORES     # 1024 batch rows per core
MT = H // P          # 8 m-tiles (hidden units)
KT = K // P          # 16 k-chunks of 128
KK = K // (2 * P)    # 8 double-chunks of 256 (DoubleRow)
NFREE = 512          # moving free dim (one PSUM bank of fp32)
NT = BS // NFREE     # 2 n-tiles per core
KO = 8               # feature chunks per 1024-feature tensor
WS = 512.0           # host-side weight scale for fp8 range

F32 = mybir.dt.float32
BF16 = mybir.dt.bfloat16
F8 = mybir.dt.float8e4

AF = mybir.ActivationFunctionType
DR = mybir.MatmulPerfMode.DoubleRow


def build_kernel(mode: str = "fp8h"):
    """Build the per-core Bass kernel. Returns compiled nc."""
    assert mode in ("fp8h", "split")
    split = mode == "split"
    nc = bacc.Bacc("TRN2", target_bir_lowering=False, debug=False)

    # DRAM I/O (per-core shapes). Activations are pre-swizzled on the host
    # to [P, NT*KO*NFREE] so each partition's bytes are contiguous.
    AW = NT * KO * NFREE
    x8 = nc.dram_tensor("x8", [P, AW], F8, kind="ExternalInput").ap()
    h8 = nc.dram_tensor("h8", [P, AW], F8, kind="ExternalInput").ap()
    hb = nc.dram_tensor("hb", [P, AW], BF16, kind="ExternalInput").ap()
    Wr = nc.dram_tensor("Wr", [MT, P, K], F8, kind="ExternalInput").ap()
    Wz = nc.dram_tensor("Wz", [MT, P, K], F8, kind="ExternalInput").ap()
    if split:
        xb = nc.dram_tensor("xb", [P, AW], BF16, kind="ExternalInput").ap()
        Whh = nc.dram_tensor("Whh", [MT, P, H], F8, kind="ExternalInput").ap()
        Whx = nc.dram_tensor("Whx", [MT, P, I], BF16,
                             kind="ExternalInput").ap()
    else:
        Wh = nc.dram_tensor("Wh", [MT, P, K], F8, kind="ExternalInput").ap()
    bz = nc.dram_tensor("bz", [P, MT], F32, kind="ExternalInput").ap()
    br = nc.dram_tensor("br", [P, MT], F32, kind="ExternalInput").ap()
    bh = nc.dram_tensor("bh", [P, MT], F32, kind="ExternalInput").ap()
    out = nc.dram_tensor("out", [H, BS], BF16, kind="ExternalOutput").ap()

    with tile.TileContext(nc) as tc:
        with (
            tc.tile_pool(name="acts", bufs=1) as acts,
            tc.tile_pool(name="gates", bufs=1) as gates,
            tc.tile_pool(name="wpool", bufs=1) as wpool,
            tc.tile_pool(name="opool", bufs=10) as opool,
            tc.tile_pool(name="ppool", bufs=8, space="PSUM") as ppool,
        ):
            bz_sb = acts.tile([P, MT], F32)
            br_sb = acts.tile([P, MT], F32)
            bh_sb = acts.tile([P, MT], F32)

            # Weight tiles, [P, KT, P]: [:, 2k:2k+2, :] is a DoubleRow
            # stationary operand [128, 2, 128].
            wr_sb = [wpool.tile([P, KT, P], F8, name=f"wr{m}")
                     for m in range(MT)]
            wz_sb = [wpool.tile([P, KT, P], F8, name=f"wz{m}")
                     for m in range(MT)]
            if split:
                whh_sb = [wpool.tile([P, KT // 2, P], F8, name=f"whh{m}")
                          for m in range(MT)]
                whx_sb = [wpool.tile([P, I], BF16, name=f"whx{m}")
                          for m in range(MT)]
            else:
                wh_sb = [wpool.tile([P, KT, P], F8, name=f"wh{m}")
                         for m in range(MT)]

            # Pre-warm the ACT sigmoid table during the DMA fill.
            warm = acts.tile([P, 1], F32)
            nc.vector.memset(warm[:], 0.0)
            nc.scalar.activation(warm[:], warm[:], AF.Sigmoid)

            # Persistent activations: [p, n-half, ko, bw]
            x8_sb = acts.tile([P, NT, KO, NFREE], F8)
            h8_sb = acts.tile([P, NT, KO, NFREE], F8)
            hb_sb = acts.tile([P, NT, KO, NFREE], BF16)
            xb_sb = (acts.tile([P, NT, KO, NFREE], BF16, name="xb_sb")
                     if split else None)

            def half(dram, n):
                return dram[:, n * KO * NFREE:(n + 1) * KO * NFREE]

            # Head DMA schedule. The h-half of the acts rides the sync ring
            # and the x-half the scalar ring, so the mt0 chains' kk0-3
            # (h_prev) and kk4-7 (x) payloads stream in parallel instead of
            # serializing on one HWDGE ring (the early window is chip-HBM
            # contended -- all 8 cores load at once). The gpsimd SWDGE queue
            # (~1us extra latency, otherwise idle) takes everything needed
            # later than ~15us: remaining weights, then hb, then wz/wh.
            q0 = 2 * NFREE  # first two feature-chunks of a half
            nc.sync.dma_start(h8_sb[:, 0, 0:2, :], h8[:, 0:q0])
            nc.sync.dma_start(h8_sb[:, 0, 2:, :], h8[:, q0:KO * NFREE])
            nc.sync.dma_start(h8_sb[:, 1], half(h8, 1))
            nc.scalar.dma_start(br_sb[:], br)
            nc.scalar.dma_start(x8_sb[:, 0], half(x8, 0))
            nc.scalar.dma_start(x8_sb[:, 1], half(x8, 1))
            nc.scalar.dma_start(bz_sb[:], bz)
            nc.scalar.dma_start(bh_sb[:], bh)
            # wr0/wr1 are needed first/soon; everything after them is bulk.
            # SDMA engines round-robin between queues at packet granularity,
            # so un-gated bulk on the gpsimd ring would steal ~1/3 of the
            # chip-contended head bandwidth from the critical act loads
            # above. The dummy copy below reads from the x8 n1 half, so the
            # tile framework holds the bulk descriptors back until the last
            # critical act DMA has landed.
            nc.gpsimd.dma_start(wr_sb[0][:], Wr[0])
            nc.gpsimd.dma_start(wr_sb[1][:], Wr[1])
            dma_gate = opool.tile([P, 8], F8, name="dma_gate")
            nc.gpsimd.tensor_copy(dma_gate[:], x8_sb[:, 0, 0, 0:8])
            for m in range(2, MT):
                nc.gpsimd.dma_start(wr_sb[m][:], Wr[m])
            for n in range(NT):
                nc.gpsimd.dma_start(hb_sb[:, n], half(hb, n))
            for m in range(MT):
                nc.gpsimd.dma_start(wz_sb[m][:], Wz[m])
            if split:
                for n in range(NT):
                    nc.scalar.dma_start(xb_sb[:, n], half(xb, n))
                for m in range(MT):
                    nc.gpsimd.dma_start(whh_sb[m][:], Whh[m])
                for m in range(MT):
                    nc.gpsimd.dma_start(whx_sb[m][:], Whx[m])
            else:
                for m in range(MT):
                    nc.gpsimd.dma_start(wh_sb[m][:], Wh[m])

            # Gate results, same swizzled layout
            z_sb = gates.tile([P, NT, KO, NFREE], BF16)
            rh_sb = gates.tile([P, NT, KO, NFREE], F8)

            # Warm the PE while the first acts stream in: ~3.4us of dummy
            # matmuls on a zeroed tile un-throttle the HAM clock gate
            # (1.2 -> 2.4 GHz needs a busy activity window), so the first
            # real chains run at full rate instead of paying the cold ramp.
            zt = acts.tile([P, 2, NFREE // 2], F8)
            nc.vector.memset(zt[:], 0.0)
            ps_wf = ppool.tile([P, NFREE], F32, tag="ps", name="ps_warm")
            for i in range(16):
                nc.tensor.matmul(ps_wf[:, 0:NFREE // 2], zt[:, :, 0:P], zt[:],
                                 start=(i == 0), stop=(i == 15), perf_mode=DR)

            def rz_rhs(kk, n):
                """fp8 moving operand [128,2,512] for concat chunk kk."""
                if kk < KK // 2:
                    return h8_sb[:, n, 2 * kk:2 * kk + 2, :]
                c = kk - KK // 2
                return x8_sb[:, n, 2 * c:2 * c + 2, :]

            def h_rhs(kk, n):
                """fp8 moving operand for the h-gate ([r*h_prev, x])."""
                if kk < KK // 2:
                    return rh_sb[:, n, 2 * kk:2 * kk + 2, :]
                c = kk - KK // 2
                return x8_sb[:, n, 2 * c:2 * c + 2, :]

            def finish(stage, mt, n, ps, width=NFREE, sub=0):
                """PSUM -> activation -> elementwise -> (store)."""
                lo, hi = sub * width, (sub + 1) * width
                if stage == "r":
                    r_tmp = opool.tile([P, width], BF16, tag="rt")
                    nc.scalar.activation(r_tmp, ps, AF.Sigmoid,
                                         bias=br_sb[:, mt:mt + 1],
                                         scale=1.0 / WS)
                    nc.vector.tensor_mul(
                        rh_sb[:, n, mt, lo:hi], r_tmp, hb_sb[:, n, mt, lo:hi])
                elif stage == "z":
                    nc.scalar.activation(z_sb[:, n, mt, lo:hi], ps,
                                         AF.Sigmoid,
                                         bias=bz_sb[:, mt:mt + 1],
                                         scale=1.0 / WS)
                else:  # h = h_prev + z*(h_tilde - h_prev)
                    hpv = hb_sb[:, n, mt, lo:hi]
                    ht = opool.tile([P, width], BF16, tag="ht")
                    nc.scalar.activation(ht, ps, AF.Tanh,
                                         bias=bh_sb[:, mt:mt + 1],
                                         scale=1.0 / WS)
                    nc.vector.tensor_sub(ht, ht, hpv)
                    nc.vector.tensor_mul(ht, ht, z_sb[:, n, mt, lo:hi])
                    nc.vector.tensor_add(ht, ht, hpv)
                    ns = slice(n * NFREE + lo, n * NFREE + hi)
                    nc.sync.dma_start(out[mt * P:(mt + 1) * P, ns], ht)

            def chain(stage, w_sb, rhs, mt, n, nsub=1, nchain=1):
                """One (mt, n) PSUM accumulation chain + its epilogue.

                LDWEIGHTS is emitted 1:1 per matmul by the compiler and at
                ~135ns hides under the ~216ns moving-port-bound DR matmul
                stream, so plain k-sequential chains already run at the
                roofline; chain order only needs to match DMA arrival order.
                nchain>1 splits the matmuls into narrower column chains so
                the epilogue of chain c pipelines under chain c+1's matmuls
                (used for the very last group to shrink the kernel tail).
                """
                wc = NFREE // nchain
                for c in range(nchain):
                    psf = ppool.tile([P, NFREE], F32, tag="ps",
                                     name=f"ps_{stage}{mt}_{n}_{c}")
                    ps = psf[:, 0:wc]
                    for kk in range(KK):
                        nc.tensor.matmul(
                            ps, w_sb[mt][:, 2 * kk:2 * kk + 2, :],
                            rhs(kk, n)[:, :, c * wc:(c + 1) * wc],
                            start=(kk == 0), stop=(kk == KK - 1),
                            perf_mode=DR)
                    w2 = wc // nsub
                    for s in range(nsub):
                        finish(stage, mt, n, ps[:, s * w2:(s + 1) * w2],
                               width=w2, sub=c * nsub + s)

            # r-gate ramp: mt0/mt1 x both batch halves as four open PSUM
            # groups, all h_prev contraction chunks first, then all x
            # chunks -- matching DMA arrival order (h8 n0, h8 n1, x8 n0,
            # x8 n1) so the PE always has runnable work while the
            # chip-contended head DMA streams the x half in.
            psr = {(mt, n): ppool.tile([P, NFREE], F32, tag="ps",
                                       name=f"ps_r{mt}_{n}")
                   for mt in (0, 1) for n in range(NT)}
            for khalf in range(2):
                for n in range(NT):
                    for kk in range(khalf * KK // 2, (khalf + 1) * KK // 2):
                        for mt in (0, 1):
                            nc.tensor.matmul(
                                psr[(mt, n)],
                                wr_sb[mt][:, 2 * kk:2 * kk + 2, :],
                                rz_rhs(kk, n),
                                start=(kk == 0), stop=(kk == KK - 1),
                                perf_mode=DR)
            for mt in (0, 1):
                for n in range(NT):
                    finish("r", mt, n, psr[(mt, n)])
            for mt in range(2, MT):
                for n in range(NT):
                    chain("r", wr_sb, rz_rhs, mt, n)
            for mt in range(MT):
                for n in range(NT):
                    chain("z", wz_sb, rz_rhs, mt, n)

            if not split:
                for mt in range(MT):
                    for n in range(NT):
                        last = mt == MT - 1 and n == NT - 1
                        chain("h", wh_sb, h_rhs, mt, n,
                              nsub=1 if last else (2 if mt == MT - 1 else 1),
                              nchain=2 if last else 1)
            else:
                for mt in range(MT):
                    for n in range(NT):
                        ps = ppool.tile([P, NFREE], F32, tag="ps",
                                        name=f"ps_h{mt}_{n}")
                        for kc in range(KT // 2):
                            nc.tensor.matmul(
                                ps, whx_sb[mt][:, kc * P:(kc + 1) * P],
                                xb_sb[:, n, kc, :],
                                start=(kc == 0), stop=False)
                        for kk in range(KK // 2):
                            nc.tensor.matmul(
                                ps, whh_sb[mt][:, 2 * kk:2 * kk + 2, :],
                                rh_sb[:, n, 2 * kk:2 * kk + 2, :],
                                start=False, stop=(kk == KK // 2 - 1),
                                perf_mode=DR)
                        last = mt == MT - 1
                        nsub = 4 if (last and n == NT - 1) else (
                            2 if last else 1)
                        w2 = NFREE // nsub
                        for s in range(nsub):
                            finish("h", mt, n, ps[:, s * w2:(s + 1) * w2],
                                   width=w2, sub=s)

    nc.compile()
    return nc


def _prep_inputs(x, h_prev, W_z, b_z, W_r, b_r, W_h, b_h, mode="fp8h"):
    """Host-side relayout: swizzled feature-major acts, m-tiled weights."""
    import ml_dtypes
    F8NP = ml_dtypes.float8_e4m3fn
    BFNP = ml_dtypes.bfloat16
    split = mode == "split"

    def prep_w(W, dt):
        # w[mt, p, ko*128+m] = W[mt*128+m, ko*128+p], scaled for fp8 range
        MTl, Kl = W.shape[0] // P, W.shape[1]
        W4 = (W * WS).reshape(MTl, P, Kl // P, P)      # [mt, m, ko, p]
        return np.ascontiguousarray(
            W4.transpose(0, 3, 2, 1)).reshape(MTl, P, Kl).astype(dt)

    def prep_act(aT, dt):
        # [F, bs] -> [p, n, ko, bw] -> flat [P, AW]
        a4 = aT.reshape(KO, P, NT, NFREE).transpose(1, 2, 0, 3)
        return np.ascontiguousarray(a4).reshape(P, NT * KO * NFREE).astype(dt)

    def prep_b(b):
        return np.ascontiguousarray(b.reshape(MT, P).T)

    xT = np.ascontiguousarray(x.T)                         # [I, B] f32
    hT = np.ascontiguousarray(h_prev.T)                    # [H, B] f32
    shared = {
        "Wr": prep_w(W_r, F8NP), "Wz": prep_w(W_z, F8NP),
        "bz": prep_b(b_z), "br": prep_b(b_r), "bh": prep_b(b_h),
    }
    if split:
        shared["Whh"] = prep_w(W_h[:, :H], F8NP)
        shared["Whx"] = prep_w(W_h[:, H:], BFNP)
    else:
        shared["Wh"] = prep_w(W_h, F8NP)
    in_maps = []
    for c in range(NCORES):
        bs = slice(c * BS, (c + 1) * BS)
        m = dict(shared)
        m["x8"] = prep_act(xT[:, bs], F8NP)
        m["h8"] = prep_act(hT[:, bs], F8NP)
        m["hb"] = prep_act(hT[:, bs], BFNP)
        if split:
            m["xb"] = prep_act(xT[:, bs], BFNP)
        in_maps.append(m)
    return in_maps


def run(inputs, mode="fp8h", trace=False, **run_kwargs):
    """Compile + run on 8 cores. Returns (output [B,H] f32, results)."""
    run_kwargs.pop("mm_dtype", None)
    nc = build_kernel(mode)
    in_maps = _prep_inputs(**inputs, mode=mode)
    res = bass_utils.run_bass_kernel_spmd(
        nc, in_maps, core_ids=list(range(NCORES)), trace=trace, **run_kwargs)
    outT = np.concatenate(
        [res.results[c]["out"] for c in range(NCORES)], axis=1)  # [H, B] bf16
    return np.ascontiguousarray(outT.T).astype(np.float32), res


def kernel(**inputs) -> np.ndarray:
    import time as _time
    try:
        out, _ = run(inputs)
    except Exception:
        # The axon-tunneled device occasionally reports a transient
        # "unrecoverable" state right after a crashed session; a fresh
        # attempt after a short pause recovers.
        _time.sleep(15)
        out, _ = run(inputs)
    return out


# revision 23
# speedup vs baseline: 1.0288x; 1.0054x over previous
"""GRU cell kernel for Trainium2, data-parallel across 8 NeuronCores.

Reference computation (per batch row):
    concat = [h_prev, x]                       # [B, 2048]
    z = sigmoid(concat @ W_z.T + b_z)          # [B, 1024]
    r = sigmoid(concat @ W_r.T + b_r)
    h_tilde = tanh([r*h_prev, x] @ W_h.T + b_h)
    h = (1-z)*h_prev + z*h_tilde

Sharding: batch dim (8192) split 1024/core; weights replicated.
Layout on device is feature-major; batch is the matmul moving dimension,
hidden units the PSUM partition dim. Host transposes in/out.

Matmuls run in fp8-e4m3 with perf_mode=DoubleRow (2 contraction rows per
PE cell). The PE moving port feeds 2 bytes/partition/cycle, so a DR
matmul streams a [256 x 512-batch] contraction chunk in ~512 cycles --
2x the flops of bf16 per cycle; measured ~216ns/MM = ~155 TF/s, the fp8
roofline. Weights are host-scaled by 512 so |w|<=11.3 sits in e4m3's
normal range (raw |w|<=0.022 is subnormal); the activation instruction's
scale operand undoes it for free.

Activations are host-swizzled to [partition, batch-half, feature-chunk,
512] so every DMA moves 4KB-contiguous runs per partition (128
descriptors/transfer instead of 1024 512B ones).

mode:
  fp8h  - all three gates fp8-DR.           (HW rel_fro ~1.76e-2)
  split - r/z fp8-DR; h-gate h-part fp8-DR over r*h_prev, x-part bf16.
                                            (sim rel_fro ~1.25e-2)
"""

import numpy as np

import concourse.bacc as bacc
import concourse.bass as bass
import concourse.mybir as mybir
import concourse.tile as tile
from concourse import bass_utils

P = 128
B = 8192
I = 1024
H = 1024
K = I + H            # 2048 contraction
NCORES = 8
BS = B // NCORES     # 1024 batch rows per core
MT = H // P          # 8 m-tiles (hidden units)
KT = K // P          # 16 k-chunks of 128
KK = K // (2 * P)    # 8 double-chunks of 256 (DoubleRow)
NFREE = 512          # moving free dim (one PSUM bank of fp32)
NT = BS // NFREE     # 2 n-tiles per core
KO = 8               # feature chunks per 1024-feature tensor
WS = 512.0           # host-side weight scale for fp8 range

F32 = mybir.dt.float32
BF16 = mybir.dt.bfloat16
F8 = mybir.dt.float8e4

AF = mybir.ActivationFunctionType
DR = mybir.MatmulPerfMode.DoubleRow


def build_kernel(mode: str = "fp8h"):
    """Build the per-core Bass kernel. Returns compiled nc."""
    assert mode in ("fp8h", "split")
    split = mode == "split"
    nc = bacc.Bacc("TRN2", target_bir_lowering=False, debug=False)

    # DRAM I/O (per-core shapes). Activations are pre-swizzled on the host
    # to [P, NT*KO*NFREE] so each partition's bytes are contiguous.
    AW = NT * KO * NFREE
    x8 = nc.dram_tensor("x8", [P, AW], F8, kind="ExternalInput").ap()
    h8 = nc.dram_tensor("h8", [P, AW], F8, kind="ExternalInput").ap()
    hb = nc.dram_tensor("hb", [P, AW], BF16, kind="ExternalInput").ap()
    Wr = nc.dram_tensor("Wr", [MT, P, K], F8, kind="ExternalInput").ap()
    Wz = nc.dram_tensor("Wz", [MT, P, K], F8, kind="ExternalInput").ap()
    if split:
        xb = nc.dram_tensor("xb", [P, AW], BF16, kind="ExternalInput").ap()
        Whh = nc.dram_tensor("Whh", [MT, P, H], F8, kind="ExternalInput").ap()
        Whx = nc.dram_tensor("Whx", [MT, P, I], BF16,
                             kind="ExternalInput").ap()
    else:
        Wh = nc.dram_tensor("Wh", [MT, P, K], F8, kind="ExternalInput").ap()
    bz = nc.dram_tensor("bz", [P, MT], F32, kind="ExternalInput").ap()
    br = nc.dram_tensor("br", [P, MT], F32, kind="ExternalInput").ap()
    bh = nc.dram_tensor("bh", [P, MT], F32, kind="ExternalInput").ap()
    out = nc.dram_tensor("out", [H, BS], BF16, kind="ExternalOutput").ap()

    with tile.TileContext(nc) as tc:
        with (
            tc.tile_pool(name="acts", bufs=1) as acts,
            tc.tile_pool(name="gates", bufs=1) as gates,
            tc.tile_pool(name="wpool", bufs=1) as wpool,
            tc.tile_pool(name="opool", bufs=10) as opool,
            tc.tile_pool(name="ppool", bufs=8, space="PSUM") as ppool,
        ):
            bz_sb = acts.tile([P, MT], F32)
            br_sb = acts.tile([P, MT], F32)
            bh_sb = acts.tile([P, MT], F32)

            # Weight tiles, [P, KT, P]: [:, 2k:2k+2, :] is a DoubleRow
            # stationary operand [128, 2, 128].
            wr_sb = [wpool.tile([P, KT, P], F8, name=f"wr{m}")
                     for m in range(MT)]
            wz_sb = [wpool.tile([P, KT, P], F8, name=f"wz{m}")
                     for m in range(MT)]
            if split:
                whh_sb = [wpool.tile([P, KT // 2, P], F8, name=f"whh{m}")
                          for m in range(MT)]
                whx_sb = [wpool.tile([P, I], BF16, name=f"whx{m}")
                          for m in range(MT)]
            else:
                wh_sb = [wpool.tile([P, KT, P], F8, name=f"wh{m}")
                         for m in range(MT)]

            # Pre-warm the ACT sigmoid table during the DMA fill.
            warm = acts.tile([P, 1], F32)
            nc.vector.memset(warm[:], 0.0)
            nc.scalar.activation(warm[:], warm[:], AF.Sigmoid)

            # Persistent activations: [p, n-half, ko, bw]
            x8_sb = acts.tile([P, NT, KO, NFREE], F8)
            h8_sb = acts.tile([P, NT, KO, NFREE], F8)
            hb_sb = acts.tile([P, NT, KO, NFREE], BF16)
            xb_sb = (acts.tile([P, NT, KO, NFREE], BF16, name="xb_sb")
                     if split else None)

            def half(dram, n):
                return dram[:, n * KO * NFREE:(n + 1) * KO * NFREE]

            # Head DMA schedule. The h-half of the acts rides the sync ring
            # and the x-half the scalar ring, so the mt0 chains' kk0-3
            # (h_prev) and kk4-7 (x) payloads stream in parallel instead of
            # serializing on one HWDGE ring (the early window is chip-HBM
            # contended -- all 8 cores load at once). The gpsimd SWDGE queue
            # (~1us extra latency, otherwise idle) takes everything needed
            # later than ~15us: remaining weights, then hb, then wz/wh.
            q0 = 2 * NFREE  # first two feature-chunks of a half
            # The Scalar sequencer spends ~2.6us on hoisted ACT table loads
            # before it can issue DMAs, so the scalar ring only carries
            # payloads needed after ~14us (the n1 halves).
            nc.sync.dma_start(h8_sb[:, 0, 0:2, :], h8[:, 0:q0])
            nc.sync.dma_start(h8_sb[:, 0, 2:, :], h8[:, q0:KO * NFREE])
            nc.sync.dma_start(x8_sb[:, 0], half(x8, 0))
            nc.scalar.dma_start(br_sb[:], br)
            nc.scalar.dma_start(h8_sb[:, 1], half(h8, 1))
            nc.scalar.dma_start(x8_sb[:, 1], half(x8, 1))
            nc.scalar.dma_start(bz_sb[:], bz)
            nc.scalar.dma_start(bh_sb[:], bh)
            # wr0/wr1 are needed first/soon; everything after them is bulk.
            # SDMA engines round-robin between queues at packet granularity,
            # so un-gated bulk on the gpsimd ring would steal ~1/3 of the
            # chip-contended head bandwidth from the critical act loads
            # above. The dummy copy below reads from the x8 n1 half, so the
            # tile framework holds the bulk descriptors back until the last
            # critical act DMA has landed.
            nc.gpsimd.dma_start(wr_sb[0][:], Wr[0])
            nc.gpsimd.dma_start(wr_sb[1][:], Wr[1])
            dma_gate = opool.tile([P, 8], F8, name="dma_gate")
            nc.gpsimd.tensor_copy(dma_gate[:], x8_sb[:, 0, 0, 0:8])
            for m in range(2, MT):
                nc.gpsimd.dma_start(wr_sb[m][:], Wr[m])
            for n in range(NT):
                nc.gpsimd.dma_start(hb_sb[:, n], half(hb, n))
            for m in range(MT):
                nc.gpsimd.dma_start(wz_sb[m][:], Wz[m])
            if split:
                for n in range(NT):
                    nc.scalar.dma_start(xb_sb[:, n], half(xb, n))
                for m in range(MT):
                    nc.gpsimd.dma_start(whh_sb[m][:], Whh[m])
                for m in range(MT):
                    nc.gpsimd.dma_start(whx_sb[m][:], Whx[m])
            else:
                for m in range(MT):
                    nc.gpsimd.dma_start(wh_sb[m][:], Wh[m])

            # Gate results, same swizzled layout
            z_sb = gates.tile([P, NT, KO, NFREE], BF16)
            rh_sb = gates.tile([P, NT, KO, NFREE], F8)

            # Warm the PE while the first acts stream in: ~3.4us of dummy
            # matmuls on a zeroed tile un-throttle the HAM clock gate
            # (1.2 -> 2.4 GHz needs a busy activity window), so the first
            # real chains run at full rate instead of paying the cold ramp.
            zt = acts.tile([P, 2, NFREE // 2], F8)
            nc.vector.memset(zt[:], 0.0)
            ps_wf = ppool.tile([P, NFREE], F32, tag="ps", name="ps_warm")
            for i in range(16):
                nc.tensor.matmul(ps_wf[:, 0:NFREE // 2], zt[:, :, 0:P], zt[:],
                                 start=(i == 0), stop=(i == 15), perf_mode=DR)

            def rz_rhs(kk, n):
                """fp8 moving operand [128,2,512] for concat chunk kk."""
                if kk < KK // 2:
                    return h8_sb[:, n, 2 * kk:2 * kk + 2, :]
                c = kk - KK // 2
                return x8_sb[:, n, 2 * c:2 * c + 2, :]

            def h_rhs(kk, n):
                """fp8 moving operand for the h-gate ([r*h_prev, x])."""
                if kk < KK // 2:
                    return rh_sb[:, n, 2 * kk:2 * kk + 2, :]
                c = kk - KK // 2
                return x8_sb[:, n, 2 * c:2 * c + 2, :]

            def finish(stage, mt, n, ps, width=NFREE, sub=0):
                """PSUM -> activation -> elementwise -> (store)."""
                lo, hi = sub * width, (sub + 1) * width
                if stage == "r":
                    r_tmp = opool.tile([P, width], BF16, tag="rt")
                    nc.scalar.activation(r_tmp, ps, AF.Sigmoid,
                                         bias=br_sb[:, mt:mt + 1],
                                         scale=1.0 / WS)
                    nc.vector.tensor_mul(
                        rh_sb[:, n, mt, lo:hi], r_tmp, hb_sb[:, n, mt, lo:hi])
                elif stage == "z":
                    nc.scalar.activation(z_sb[:, n, mt, lo:hi], ps,
                                         AF.Sigmoid,
                                         bias=bz_sb[:, mt:mt + 1],
                                         scale=1.0 / WS)
                else:  # h = h_prev + z*(h_tilde - h_prev)
                    hpv = hb_sb[:, n, mt, lo:hi]
                    ht = opool.tile([P, width], BF16, tag="ht")
                    nc.scalar.activation(ht, ps, AF.Tanh,
                                         bias=bh_sb[:, mt:mt + 1],
                                         scale=1.0 / WS)
                    nc.vector.tensor_sub(ht, ht, hpv)
                    nc.vector.tensor_mul(ht, ht, z_sb[:, n, mt, lo:hi])
                    nc.vector.tensor_add(ht, ht, hpv)
                    ns = slice(n * NFREE + lo, n * NFREE + hi)
                    nc.sync.dma_start(out[mt * P:(mt + 1) * P, ns], ht)

            def chain(stage, w_sb, rhs, mt, n, nsub=1, nchain=1):
                """One (mt, n) PSUM accumulation chain + its epilogue.

                LDWEIGHTS is emitted 1:1 per matmul by the compiler and at
                ~135ns hides under the ~216ns moving-port-bound DR matmul
                stream, so plain k-sequential chains already run at the
                roofline; chain order only needs to match DMA arrival order.
                nchain>1 splits the matmuls into narrower column chains so
                the epilogue of chain c pipelines under chain c+1's matmuls
                (used for the very last group to shrink the kernel tail).
                """
                wc = NFREE // nchain
                for c in range(nchain):
                    psf = ppool.tile([P, NFREE], F32, tag="ps",
                                     name=f"ps_{stage}{mt}_{n}_{c}")
                    ps = psf[:, 0:wc]
                    for kk in range(KK):
                        nc.tensor.matmul(
                            ps, w_sb[mt][:, 2 * kk:2 * kk + 2, :],
                            rhs(kk, n)[:, :, c * wc:(c + 1) * wc],
                            start=(kk == 0), stop=(kk == KK - 1),
                            perf_mode=DR)
                    w2 = wc // nsub
                    for s in range(nsub):
                        finish(stage, mt, n, ps[:, s * w2:(s + 1) * w2],
                               width=w2, sub=c * nsub + s)

            # r-gate ramp: mt0/mt1 x both batch halves as four open PSUM
            # groups, all h_prev contraction chunks first, then all x
            # chunks -- matching DMA arrival order (h8 n0, h8 n1, x8 n0,
            # x8 n1) so the PE always has runnable work while the
            # chip-contended head DMA streams the x half in.
            psr = {(mt, n): ppool.tile([P, NFREE], F32, tag="ps",
                                       name=f"ps_r{mt}_{n}")
                   for mt in (0, 1) for n in range(NT)}
            for khalf in range(2):
                for n in range(NT):
                    for kk in range(khalf * KK // 2, (khalf + 1) * KK // 2):
                        for mt in (0, 1):
                            nc.tensor.matmul(
                                psr[(mt, n)],
                                wr_sb[mt][:, 2 * kk:2 * kk + 2, :],
                                rz_rhs(kk, n),
                                start=(kk == 0), stop=(kk == KK - 1),
                                perf_mode=DR)
            for mt in (0, 1):
                for n in range(NT):
                    finish("r", mt, n, psr[(mt, n)])
            for mt in range(2, MT):
                for n in range(NT):
                    chain("r", wr_sb, rz_rhs, mt, n)
            # z gate: chains interleaved in pairs (consecutive matmuls
            # alternate between two PSUM banks).
            for mt in range(0, MT, 2):
                psz = {(m, n): ppool.tile([P, NFREE], F32, tag="ps",
                                          name=f"ps_z{m}_{n}")
                       for m in (mt, mt + 1) for n in range(NT)}
                for n in range(NT):
                    for kk in range(KK):
                        for m in (mt, mt + 1):
                            nc.tensor.matmul(
                                psz[(m, n)],
                                wz_sb[m][:, 2 * kk:2 * kk + 2, :],
                                rz_rhs(kk, n),
                                start=(kk == 0), stop=(kk == KK - 1),
                                perf_mode=DR)
                for m in (mt, mt + 1):
                    for n in range(NT):
                        finish("z", m, n, psz[(m, n)])

            if not split:
                for mt in range(MT):
                    for n in range(NT):
                        last = mt == MT - 1 and n == NT - 1
                        chain("h", wh_sb, h_rhs, mt, n,
                              nsub=1 if last else (2 if mt == MT - 1 else 1),
                              nchain=2 if last else 1)
            else:
                for mt in range(MT):
                    for n in range(NT):
                        ps = ppool.tile([P, NFREE], F32, tag="ps",
                                        name=f"ps_h{mt}_{n}")
                        for kc in range(KT // 2):
                            nc.tensor.matmul(
                                ps, whx_sb[mt][:, kc * P:(kc + 1) * P],
                                xb_sb[:, n, kc, :],
                                start=(kc == 0), stop=False)
                        for kk in range(KK // 2):
                            nc.tensor.matmul(
                                ps, whh_sb[mt][:, 2 * kk:2 * kk + 2, :],
                                rh_sb[:, n, 2 * kk:2 * kk + 2, :],
                                start=False, stop=(kk == KK // 2 - 1),
                                perf_mode=DR)
                        last = mt == MT - 1
                        nsub = 4 if (last and n == NT - 1) else (
                            2 if last else 1)
                        w2 = NFREE // nsub
                        for s in range(nsub):
                            finish("h", mt, n, ps[:, s * w2:(s + 1) * w2],
                                   width=w2, sub=s)

    nc.compile()
    return nc


def _prep_inputs(x, h_prev, W_z, b_z, W_r, b_r, W_h, b_h, mode="fp8h"):
    """Host-side relayout: swizzled feature-major acts, m-tiled weights."""
    import ml_dtypes
    F8NP = ml_dtypes.float8_e4m3fn
    BFNP = ml_dtypes.bfloat16
    split = mode == "split"

    def prep_w(W, dt):
        # w[mt, p, ko*128+m] = W[mt*128+m, ko*128+p], scaled for fp8 range
        MTl, Kl = W.shape[0] // P, W.shape[1]
        W4 = (W * WS).reshape(MTl, P, Kl // P, P)      # [mt, m, ko, p]
        return np.ascontiguousarray(
            W4.transpose(0, 3, 2, 1)).reshape(MTl, P, Kl).astype(dt)

    def prep_act(aT, dt):
        # [F, bs] -> [p, n, ko, bw] -> flat [P, AW]
        a4 = aT.reshape(KO, P, NT, NFREE).transpose(1, 2, 0, 3)
        return np.ascontiguousarray(a4).reshape(P, NT * KO * NFREE).astype(dt)

    def prep_b(b):
        return np.ascontiguousarray(b.reshape(MT, P).T)

    xT = np.ascontiguousarray(x.T)                         # [I, B] f32
    hT = np.ascontiguousarray(h_prev.T)                    # [H, B] f32
    shared = {
        "Wr": prep_w(W_r, F8NP), "Wz": prep_w(W_z, F8NP),
        "bz": prep_b(b_z), "br": prep_b(b_r), "bh": prep_b(b_h),
    }
    if split:
        shared["Whh"] = prep_w(W_h[:, :H], F8NP)
        shared["Whx"] = prep_w(W_h[:, H:], BFNP)
    else:
        shared["Wh"] = prep_w(W_h, F8NP)
    in_maps = []
    for c in range(NCORES):
        bs = slice(c * BS, (c + 1) * BS)
        m = dict(shared)
        m["x8"] = prep_act(xT[:, bs], F8NP)
        m["h8"] = prep_act(hT[:, bs], F8NP)
        m["hb"] = prep_act(hT[:, bs], BFNP)
        if split:
            m["xb"] = prep_act(xT[:, bs], BFNP)
        in_maps.append(m)
    return in_maps


def run(inputs, mode="fp8h", trace=False, **run_kwargs):
    """Compile + run on 8 cores. Returns (output [B,H] f32, results)."""
    run_kwargs.pop("mm_dtype", None)
    nc = build_kernel(mode)
    in_maps = _prep_inputs(**inputs, mode=mode)
    res = bass_utils.run_bass_kernel_spmd(
        nc, in_maps, core_ids=list(range(NCORES)), trace=trace, **run_kwargs)
    outT = np.concatenate(
        [res.results[c]["out"] for c in range(NCORES)], axis=1)  # [H, B] bf16
    return np.ascontiguousarray(outT.T).astype(np.float32), res


def kernel(**inputs) -> np.ndarray:
    import time as _time
    try:
        out, _ = run(inputs)
    except Exception:
        # The axon-tunneled device occasionally reports a transient
        # "unrecoverable" state right after a crashed session; a fresh
        # attempt after a short pause recovers.
        _time.sleep(15)
        out, _ = run(inputs)
    return out


# revision 27
# speedup vs baseline: 1.0389x; 1.0098x over previous
"""GRU cell kernel for Trainium2, data-parallel across 8 NeuronCores.

Reference computation (per batch row):
    concat = [h_prev, x]                       # [B, 2048]
    z = sigmoid(concat @ W_z.T + b_z)          # [B, 1024]
    r = sigmoid(concat @ W_r.T + b_r)
    h_tilde = tanh([r*h_prev, x] @ W_h.T + b_h)
    h = (1-z)*h_prev + z*h_tilde

Sharding: batch dim (8192) split 1024/core; weights replicated.
Layout on device is feature-major; batch is the matmul moving dimension,
hidden units the PSUM partition dim. Host transposes in/out.

Matmuls run in fp8-e4m3 with perf_mode=DoubleRow (2 contraction rows per
PE cell). The PE moving port feeds 2 bytes/partition/cycle, so a DR
matmul streams a [256 x 512-batch] contraction chunk in ~512 cycles --
2x the flops of bf16 per cycle; measured ~216ns/MM = ~155 TF/s, the fp8
roofline. Weights are host-scaled by 512 so |w|<=11.3 sits in e4m3's
normal range (raw |w|<=0.022 is subnormal); the activation instruction's
scale operand undoes it for free.

Activations are host-swizzled to [partition, batch-half, feature-chunk,
512] so every DMA moves 4KB-contiguous runs per partition (128
descriptors/transfer instead of 1024 512B ones).

mode:
  fp8h  - all three gates fp8-DR.           (HW rel_fro ~1.76e-2)
  split - r/z fp8-DR; h-gate h-part fp8-DR over r*h_prev, x-part bf16.
                                            (sim rel_fro ~1.25e-2)
"""

import numpy as np

import concourse.bacc as bacc
import concourse.bass as bass
import concourse.mybir as mybir
import concourse.tile as tile
from concourse import bass_utils

P = 128
B = 8192
I = 1024
H = 1024
K = I + H            # 2048 contraction
NCORES = 8
BS = B // NCORES     # 1024 batch rows per core
MT = H // P          # 8 m-tiles (hidden units)
KT = K // P          # 16 k-chunks of 128
KK = K // (2 * P)    # 8 double-chunks of 256 (DoubleRow)
NFREE = 512          # moving free dim (one PSUM bank of fp32)
NT = BS // NFREE     # 2 n-tiles per core
KO = 8               # feature chunks per 1024-feature tensor
WS = 512.0           # host-side weight scale for fp8 range

F32 = mybir.dt.float32
BF16 = mybir.dt.bfloat16
F8 = mybir.dt.float8e4

AF = mybir.ActivationFunctionType
DR = mybir.MatmulPerfMode.DoubleRow


def build_kernel(mode: str = "fp8h"):
    """Build the per-core Bass kernel. Returns compiled nc."""
    assert mode in ("fp8h", "split")
    split = mode == "split"
    nc = bacc.Bacc("TRN2", target_bir_lowering=False, debug=False)

    # DRAM I/O (per-core shapes). Activations are pre-swizzled on the host
    # to [P, NT*KO*NFREE] so each partition's bytes are contiguous.
    AW = NT * KO * NFREE
    x8 = nc.dram_tensor("x8", [P, AW], F8, kind="ExternalInput").ap()
    h8 = nc.dram_tensor("h8", [P, AW], F8, kind="ExternalInput").ap()
    hb = nc.dram_tensor("hb", [P, AW], BF16, kind="ExternalInput").ap()
    Wr = nc.dram_tensor("Wr", [MT, P, K], F8, kind="ExternalInput").ap()
    Wz = nc.dram_tensor("Wz", [MT, P, K], F8, kind="ExternalInput").ap()
    if split:
        xb = nc.dram_tensor("xb", [P, AW], BF16, kind="ExternalInput").ap()
        Whh = nc.dram_tensor("Whh", [MT, P, H], F8, kind="ExternalInput").ap()
        Whx = nc.dram_tensor("Whx", [MT, P, I], BF16,
                             kind="ExternalInput").ap()
    else:
        Wh = nc.dram_tensor("Wh", [MT, P, K], F8, kind="ExternalInput").ap()
    bz = nc.dram_tensor("bz", [P, MT], F32, kind="ExternalInput").ap()
    br = nc.dram_tensor("br", [P, MT], F32, kind="ExternalInput").ap()
    bh = nc.dram_tensor("bh", [P, MT], F32, kind="ExternalInput").ap()
    out = nc.dram_tensor("out", [H, BS], BF16, kind="ExternalOutput").ap()

    with tile.TileContext(nc) as tc:
        with (
            tc.tile_pool(name="acts", bufs=1) as acts,
            tc.tile_pool(name="gates", bufs=1) as gates,
            tc.tile_pool(name="wpool", bufs=1) as wpool,
            tc.tile_pool(name="opool", bufs=10) as opool,
            tc.tile_pool(name="ppool", bufs=8, space="PSUM") as ppool,
        ):
            bz_sb = acts.tile([P, MT], F32)
            br_sb = acts.tile([P, MT], F32)
            bh_sb = acts.tile([P, MT], F32)

            # Weight tiles, [P, KT, P]: [:, 2k:2k+2, :] is a DoubleRow
            # stationary operand [128, 2, 128].
            wr_sb = [wpool.tile([P, KT, P], F8, name=f"wr{m}")
                     for m in range(MT)]
            wz_sb = [wpool.tile([P, KT, P], F8, name=f"wz{m}")
                     for m in range(MT)]
            if split:
                whh_sb = [wpool.tile([P, KT // 2, P], F8, name=f"whh{m}")
                          for m in range(MT)]
                whx_sb = [wpool.tile([P, I], BF16, name=f"whx{m}")
                          for m in range(MT)]
            else:
                wh_sb = [wpool.tile([P, KT, P], F8, name=f"wh{m}")
                         for m in range(MT)]

            # Pre-warm the ACT sigmoid table during the DMA fill.
            warm = acts.tile([P, 1], F32)
            nc.vector.memset(warm[:], 0.0)
            nc.scalar.activation(warm[:], warm[:], AF.Sigmoid)

            # Persistent activations: [p, n-half, ko, bw]
            x8_sb = acts.tile([P, NT, KO, NFREE], F8)
            h8_sb = acts.tile([P, NT, KO, NFREE], F8)
            hb_sb = acts.tile([P, NT, KO, NFREE], BF16)
            xb_sb = (acts.tile([P, NT, KO, NFREE], BF16, name="xb_sb")
                     if split else None)

            def half(dram, n):
                return dram[:, n * KO * NFREE:(n + 1) * KO * NFREE]

            # Head DMA schedule. The h-half of the acts rides the sync ring
            # and the x-half the scalar ring, so the mt0 chains' kk0-3
            # (h_prev) and kk4-7 (x) payloads stream in parallel instead of
            # serializing on one HWDGE ring (the early window is chip-HBM
            # contended -- all 8 cores load at once). The gpsimd SWDGE queue
            # (~1us extra latency, otherwise idle) takes everything needed
            # later than ~15us: remaining weights, then hb, then wz/wh.
            q0 = 2 * NFREE  # first two feature-chunks of a half
            # The Scalar sequencer spends ~2.6us on hoisted ACT table loads
            # before it can issue DMAs, so the scalar ring only carries
            # payloads needed after ~14us (the n1 halves).
            nc.sync.dma_start(h8_sb[:, 0, 0:2, :], h8[:, 0:q0])
            nc.sync.dma_start(h8_sb[:, 0, 2:, :], h8[:, q0:KO * NFREE])
            nc.sync.dma_start(x8_sb[:, 0], half(x8, 0))
            nc.scalar.dma_start(br_sb[:], br)
            nc.scalar.dma_start(h8_sb[:, 1], half(h8, 1))
            nc.scalar.dma_start(x8_sb[:, 1], half(x8, 1))
            nc.scalar.dma_start(bz_sb[:], bz)
            nc.scalar.dma_start(bh_sb[:], bh)
            # wr0/wr1 are needed first/soon; everything after them is bulk.
            # SDMA engines round-robin between queues at packet granularity,
            # so un-gated bulk on the gpsimd ring would steal ~1/3 of the
            # chip-contended head bandwidth from the critical act loads
            # above. The dummy copy below reads from the x8 n1 half, so the
            # tile framework holds the bulk descriptors back until the last
            # critical act DMA has landed.
            nc.gpsimd.dma_start(wr_sb[0][:], Wr[0])
            nc.gpsimd.dma_start(wr_sb[1][:], Wr[1])
            dma_gate = opool.tile([P, 8], F8, name="dma_gate")
            nc.gpsimd.tensor_copy(dma_gate[:], x8_sb[:, 0, 0, 0:8])
            for m in range(2, MT):
                nc.gpsimd.dma_start(wr_sb[m][:], Wr[m])
            for n in range(NT):
                nc.gpsimd.dma_start(hb_sb[:, n], half(hb, n))
            for m in range(MT):
                nc.gpsimd.dma_start(wz_sb[m][:], Wz[m])
            if split:
                for n in range(NT):
                    nc.scalar.dma_start(xb_sb[:, n], half(xb, n))
                for m in range(MT):
                    nc.gpsimd.dma_start(whh_sb[m][:], Whh[m])
                for m in range(MT):
                    nc.gpsimd.dma_start(whx_sb[m][:], Whx[m])
            else:
                for m in range(MT):
                    nc.gpsimd.dma_start(wh_sb[m][:], Wh[m])

            # Gate results, same swizzled layout
            z_sb = gates.tile([P, NT, KO, NFREE], BF16)
            rh_sb = gates.tile([P, NT, KO, NFREE], F8)

            # Warm the PE while the first acts stream in: ~3.4us of dummy
            # matmuls on a zeroed tile un-throttle the HAM clock gate
            # (1.2 -> 2.4 GHz needs a busy activity window), so the first
            # real chains run at full rate instead of paying the cold ramp.
            # ~3.5us of dummy matmuls: the HAM clock gate needs a full busy
            # activity window (~3.4us) to un-throttle 1.2 -> 2.4 GHz.
            zt = acts.tile([P, 2, NFREE // 2], F8)
            nc.vector.memset(zt[:], 0.0)
            ps_wf = ppool.tile([P, NFREE], F32, tag="ps", name="ps_warm")
            NWARM = 32
            for i in range(NWARM):
                nc.tensor.matmul(ps_wf[:, 0:NFREE // 2], zt[:, :, 0:P], zt[:],
                                 start=(i == 0), stop=(i == NWARM - 1),
                                 perf_mode=DR)

            def rz_rhs(kk, n):
                """fp8 moving operand [128,2,512] for concat chunk kk."""
                if kk < KK // 2:
                    return h8_sb[:, n, 2 * kk:2 * kk + 2, :]
                c = kk - KK // 2
                return x8_sb[:, n, 2 * c:2 * c + 2, :]

            def h_rhs(kk, n):
                """fp8 moving operand for the h-gate ([r*h_prev, x])."""
                if kk < KK // 2:
                    return rh_sb[:, n, 2 * kk:2 * kk + 2, :]
                c = kk - KK // 2
                return x8_sb[:, n, 2 * c:2 * c + 2, :]

            def finish(stage, mt, n, ps, width=NFREE, sub=0):
                """PSUM -> activation -> elementwise -> (store)."""
                lo, hi = sub * width, (sub + 1) * width
                if stage == "r":
                    r_tmp = opool.tile([P, width], BF16, tag="rt")
                    nc.scalar.activation(r_tmp, ps, AF.Sigmoid,
                                         bias=br_sb[:, mt:mt + 1],
                                         scale=1.0 / WS)
                    nc.vector.tensor_mul(
                        rh_sb[:, n, mt, lo:hi], r_tmp, hb_sb[:, n, mt, lo:hi])
                elif stage == "z":
                    nc.scalar.activation(z_sb[:, n, mt, lo:hi], ps,
                                         AF.Sigmoid,
                                         bias=bz_sb[:, mt:mt + 1],
                                         scale=1.0 / WS)
                else:
                    # device computes dh = z*(h_tilde - h_prev); the final
                    # h = h_prev + dh runs on the host in fp32 (free, and
                    # keeps the dominant h_prev term exact).
                    ht = opool.tile([P, width], BF16, tag="ht")
                    nc.scalar.activation(ht, ps, AF.Tanh,
                                         bias=bh_sb[:, mt:mt + 1],
                                         scale=1.0 / WS)
                    nc.vector.tensor_sub(ht, ht, hb_sb[:, n, mt, lo:hi])
                    nc.vector.tensor_mul(ht, ht, z_sb[:, n, mt, lo:hi])
                    ns = slice(n * NFREE + lo, n * NFREE + hi)
                    nc.sync.dma_start(out[mt * P:(mt + 1) * P, ns], ht)

            def chain(stage, w_sb, rhs, mt, n, nsub=1, nchain=1):
                """One (mt, n) PSUM accumulation chain + its epilogue.

                LDWEIGHTS is emitted 1:1 per matmul by the compiler and at
                ~135ns hides under the ~216ns moving-port-bound DR matmul
                stream, so plain k-sequential chains already run at the
                roofline; chain order only needs to match DMA arrival order.
                nchain>1 splits the matmuls into narrower column chains so
                the epilogue of chain c pipelines under chain c+1's matmuls
                (used for the very last group to shrink the kernel tail).
                """
                wc = NFREE // nchain
                for c in range(nchain):
                    psf = ppool.tile([P, NFREE], F32, tag="ps",
                                     name=f"ps_{stage}{mt}_{n}_{c}")
                    ps = psf[:, 0:wc]
                    for kk in range(KK):
                        nc.tensor.matmul(
                            ps, w_sb[mt][:, 2 * kk:2 * kk + 2, :],
                            rhs(kk, n)[:, :, c * wc:(c + 1) * wc],
                            start=(kk == 0), stop=(kk == KK - 1),
                            perf_mode=DR)
                    w2 = wc // nsub
                    for s in range(nsub):
                        finish(stage, mt, n, ps[:, s * w2:(s + 1) * w2],
                               width=w2, sub=c * nsub + s)

            # r-gate ramp: mt0/mt1 x both batch halves as four open PSUM
            # groups, all h_prev contraction chunks first, then all x
            # chunks -- matching DMA arrival order (h8 n0, h8 n1, x8 n0,
            # x8 n1) so the PE always has runnable work while the
            # chip-contended head DMA streams the x half in.
            psr = {(mt, n): ppool.tile([P, NFREE], F32, tag="ps",
                                       name=f"ps_r{mt}_{n}")
                   for mt in (0, 1) for n in range(NT)}
            for khalf in range(2):
                for n in range(NT):
                    for kk in range(khalf * KK // 2, (khalf + 1) * KK // 2):
                        for mt in (0, 1):
                            nc.tensor.matmul(
                                psr[(mt, n)],
                                wr_sb[mt][:, 2 * kk:2 * kk + 2, :],
                                rz_rhs(kk, n),
                                start=(kk == 0), stop=(kk == KK - 1),
                                perf_mode=DR)
            for mt in (0, 1):
                for n in range(NT):
                    finish("r", mt, n, psr[(mt, n)])
            for mt in range(2, MT):
                for n in range(NT):
                    chain("r", wr_sb, rz_rhs, mt, n)
            # z gate: chains interleaved in pairs (consecutive matmuls
            # alternate between two PSUM banks).
            for mt in range(0, MT, 2):
                psz = {(m, n): ppool.tile([P, NFREE], F32, tag="ps",
                                          name=f"ps_z{m}_{n}")
                       for m in (mt, mt + 1) for n in range(NT)}
                for n in range(NT):
                    for kk in range(KK):
                        for m in (mt, mt + 1):
                            nc.tensor.matmul(
                                psz[(m, n)],
                                wz_sb[m][:, 2 * kk:2 * kk + 2, :],
                                rz_rhs(kk, n),
                                start=(kk == 0), stop=(kk == KK - 1),
                                perf_mode=DR)
                for m in (mt, mt + 1):
                    for n in range(NT):
                        finish("z", m, n, psz[(m, n)])

            if not split:
                for mt in range(MT):
                    for n in range(NT):
                        chain("h", wh_sb, h_rhs, mt, n,
                              nchain=2 if mt == MT - 1 else 1)
            else:
                for mt in range(MT):
                    for n in range(NT):
                        ps = ppool.tile([P, NFREE], F32, tag="ps",
                                        name=f"ps_h{mt}_{n}")
                        for kc in range(KT // 2):
                            nc.tensor.matmul(
                                ps, whx_sb[mt][:, kc * P:(kc + 1) * P],
                                xb_sb[:, n, kc, :],
                                start=(kc == 0), stop=False)
                        for kk in range(KK // 2):
                            nc.tensor.matmul(
                                ps, whh_sb[mt][:, 2 * kk:2 * kk + 2, :],
                                rh_sb[:, n, 2 * kk:2 * kk + 2, :],
                                start=False, stop=(kk == KK // 2 - 1),
                                perf_mode=DR)
                        last = mt == MT - 1
                        nsub = 4 if (last and n == NT - 1) else (
                            2 if last else 1)
                        w2 = NFREE // nsub
                        for s in range(nsub):
                            finish("h", mt, n, ps[:, s * w2:(s + 1) * w2],
                                   width=w2, sub=s)

    nc.compile()
    return nc


def _prep_inputs(x, h_prev, W_z, b_z, W_r, b_r, W_h, b_h, mode="fp8h"):
    """Host-side relayout: swizzled feature-major acts, m-tiled weights."""
    import ml_dtypes
    F8NP = ml_dtypes.float8_e4m3fn
    BFNP = ml_dtypes.bfloat16
    split = mode == "split"

    def prep_w(W, dt):
        # w[mt, p, ko*128+m] = W[mt*128+m, ko*128+p], scaled for fp8 range
        MTl, Kl = W.shape[0] // P, W.shape[1]
        W4 = (W * WS).reshape(MTl, P, Kl // P, P)      # [mt, m, ko, p]
        return np.ascontiguousarray(
            W4.transpose(0, 3, 2, 1)).reshape(MTl, P, Kl).astype(dt)

    def prep_act(aT, dt):
        # [F, bs] -> [p, n, ko, bw] -> flat [P, AW]
        a4 = aT.reshape(KO, P, NT, NFREE).transpose(1, 2, 0, 3)
        return np.ascontiguousarray(a4).reshape(P, NT * KO * NFREE).astype(dt)

    def prep_b(b):
        return np.ascontiguousarray(b.reshape(MT, P).T)

    xT = np.ascontiguousarray(x.T)                         # [I, B] f32
    hT = np.ascontiguousarray(h_prev.T)                    # [H, B] f32
    shared = {
        "Wr": prep_w(W_r, F8NP), "Wz": prep_w(W_z, F8NP),
        "bz": prep_b(b_z), "br": prep_b(b_r), "bh": prep_b(b_h),
    }
    if split:
        shared["Whh"] = prep_w(W_h[:, :H], F8NP)
        shared["Whx"] = prep_w(W_h[:, H:], BFNP)
    else:
        shared["Wh"] = prep_w(W_h, F8NP)
    in_maps = []
    for c in range(NCORES):
        bs = slice(c * BS, (c + 1) * BS)
        m = dict(shared)
        m["x8"] = prep_act(xT[:, bs], F8NP)
        m["h8"] = prep_act(hT[:, bs], F8NP)
        m["hb"] = prep_act(hT[:, bs], BFNP)
        if split:
            m["xb"] = prep_act(xT[:, bs], BFNP)
        in_maps.append(m)
    return in_maps


def run(inputs, mode="fp8h", trace=False, **run_kwargs):
    """Compile + run on 8 cores. Returns (output [B,H] f32, results)."""
    run_kwargs.pop("mm_dtype", None)
    nc = build_kernel(mode)
    in_maps = _prep_inputs(**inputs, mode=mode)
    res = bass_utils.run_bass_kernel_spmd(
        nc, in_maps, core_ids=list(range(NCORES)), trace=trace, **run_kwargs)
    dhT = np.concatenate(
        [res.results[c]["out"] for c in range(NCORES)], axis=1)  # [H, B] bf16
    dh = np.ascontiguousarray(dhT.T).astype(np.float32)
    return inputs["h_prev"] + dh, res


def kernel(**inputs) -> np.ndarray:
    import time as _time
    try:
        out, _ = run(inputs)
    except Exception:
        # The axon-tunneled device occasionally reports a transient
        # "unrecoverable" state right after a crashed session; a fresh
        # attempt after a short pause recovers.
        _time.sleep(15)
        out, _ = run(inputs)
    return out
